# revision 3
# baseline (speedup 1.0000x reference)
"""Trainium2 Bass kernel for nn_BaseLUTLayer (soft-LUT layer).

Math: out[b,o] = sum_k lut[o,k] * prod_j (bit_j(k) ? x[b,m(o,j)] : 1-x[b,m(o,j)])

v3 strategy (Mobius / multilinear-polynomial basis):
  * Host re-parameterizes the LUT into multilinear coefficients c[o, :]
    (per-bit transform (A,B) -> (A, B-A)), so that
        out[b,o] = sum_m c[o,m] * prod_{j: bit_j(m)} x[b, map(o,j)]
    evaluated by a 6-level halving tree with ADJACENT pairing:
        t_l[i] = t_{l-1}[2i] + t_{l-1}[2i+1] * x_{map(o, l-1)}
    No 1-x / reciprocals / w-product; intermediates bounded by sum|c|
    (~500) so the whole pipeline runs in fp16.
  * Sharding: 4-way over nodes x 2-way over batch -> per core B=512
    batch rows, 512 nodes = 4 chunks of 128 nodes (nodes on partitions).
  * Gather: host passes x^T as [1024, 512] fp16 DRAM; dma_gather pulls
    6x128 rows (1KB) per chunk. No on-device transposes.
  * Engine split per chunk:
      - level 1 (32 kp-slices of scalar FMA): KACT slices on ScalarE
        (activation scale/bias), rest on DVE 2x via dup-pair trick
      - level 2 mult on DVE 2x; level-2 ADD on TensorE (identity matmul
        PSUM accumulation, fp16 moving)
      - level 3/4 on DVE (level 3 reads fp32 PSUM at 1x)
      - levels 5/6 on GpSimd (otherwise idle after gathers)
"""

import numpy as np

import concourse.bass as bass
import concourse.mybir as mybir
from concourse import bacc
from concourse import tile
from concourse.masks import make_identity
from concourse.bass_utils import run_bass_kernel_spmd

P = 128
IN = 1024
OUT = 2048
NB = 6
B_FULL = 1024
N_CORES = 8

NODE_SHARDS = 4
BATCH_SHARDS = 2
B = B_FULL // BATCH_SHARDS          # 512 batch rows per core
NODES = OUT // NODE_SHARDS          # 512 nodes per core
OHI = NODES // P                    # 4 chunks of 128 nodes

F16 = mybir.dt.float16
F32 = mybir.dt.float32
I16 = mybir.dt.int16

KACT = 11                           # level-1 kp slices per half on ScalarE
KDVE = 16 - KACT                    # remainder on DVE
IDXC = NB * P // 16                 # gidx columns per chunk (48)


def build_program():
    nc = bacc.Bacc("TRN2", target_bir_lowering=False, debug=False)

    gsrc = nc.dram_tensor("gsrc", [IN, B], F16, kind="ExternalInput").ap()
    gidx = nc.dram_tensor("gidx", [P, OHI * IDXC], I16, kind="ExternalInput").ap()
    c0f = nc.dram_tensor("c0f", [P, OHI, 32], F32, kind="ExternalInput").ap()
    c1f = nc.dram_tensor("c1f", [P, OHI, 32], F32, kind="ExternalInput").ap()
    c0d = nc.dram_tensor("c0d", [P, OHI, 32, 2], F16, kind="ExternalInput").ap()
    c1d = nc.dram_tensor("c1d", [P, OHI, 32, 2], F16, kind="ExternalInput").ap()
    outs = nc.dram_tensor("outs", [P, OHI, B], F16, kind="ExternalOutput").ap()

    ident_fn = mybir.ActivationFunctionType.Identity

    with tile.TileContext(nc) as tc:
        with (
            tc.tile_pool(name="consts", bufs=1) as consts,
            tc.tile_pool(name="zpool", bufs=4) as zpool,
            tc.tile_pool(name="t1pool", bufs=2) as t1pool,
            tc.tile_pool(name="tpool", bufs=2) as tpool,
            tc.tile_pool(name="psum", bufs=1, space="PSUM") as psum,
        ):
            gidx_sb = consts.tile([P, OHI * IDXC], I16)
            nc.sync.dma_start(gidx_sb, gidx)
            c0f_sb = consts.tile([P, OHI, 32], F32)
            nc.sync.dma_start(c0f_sb, c0f)
            c1f_sb = consts.tile([P, OHI, 32], F32)
            nc.sync.dma_start(c1f_sb, c1f)
            c0d_sb = consts.tile([P, OHI, 32, 2], F16)
            nc.sync.dma_start(c0d_sb, c0d)
            c1d_sb = consts.tile([P, OHI, 32, 2], F16)
            nc.sync.dma_start(c1d_sb, c1d)
            ident = consts.tile([P, P], F16)
            make_identity(nc, ident)

            stash = {}

            def stage_a(c):
                z = zpool.tile([P, NB, B], F16, tag="z")
                if c == 0:
                    # slot-0 rows first so level 1 can start sooner
                    nc.gpsimd.dma_gather(
                        out_ap=z[:, 0:1, :],
                        in_ap=gsrc,
                        idxs_ap=gidx_sb[:, 0 : IDXC // NB],
                        num_idxs=P,
                        num_idxs_reg=P,
                        elem_size=B,
                    )
                    nc.gpsimd.dma_gather(
                        out_ap=z[:, 1:NB, :],
                        in_ap=gsrc,
                        idxs_ap=gidx_sb[:, IDXC // NB : IDXC],
                        num_idxs=(NB - 1) * P,
                        num_idxs_reg=(NB - 1) * P,
                        elem_size=B,
                    )
                else:
                    nc.gpsimd.dma_gather(
                        out_ap=z,
                        in_ap=gsrc,
                        idxs_ap=gidx_sb[:, c * IDXC : (c + 1) * IDXC],
                        num_idxs=NB * P,
                        num_idxs_reg=NB * P,
                        elem_size=B,
                    )
                z0 = z[:, 0, :]
                # level 1: t1[o, kp, b] = c0[o,kp] + c1[o,kp] * z0[o,b]
                t1 = t1pool.tile([P, 32, B], F16, tag="t1")
                z0d = z0.rearrange("p (b2 two) -> p b2 two", two=2)
                for h in range(2):
                    k0 = h * 16
                    kd0 = k0 + KACT  # DVE takes the tail KDVE slices
                    t1v = t1[:, kd0 : k0 + 16, :].rearrange(
                        "p k (b2 two) -> p k b2 two", two=2
                    )
                    in_z = z0d[:, None, :, :].broadcast_to([P, KDVE, B // 2, 2])
                    nc.vector.tensor_mul(
                        t1v,
                        in_z,
                        c1d_sb[:, c, kd0 : k0 + 16, None, :].broadcast_to(
                            [P, KDVE, B // 2, 2]
                        ),
                    )
                    nc.vector.tensor_add(
                        t1v,
                        t1v,
                        c0d_sb[:, c, kd0 : k0 + 16, None, :].broadcast_to(
                            [P, KDVE, B // 2, 2]
                        ),
                    )
                    for kp in range(k0, kd0):
                        nc.scalar.activation(
                            t1[:, kp, :],
                            z0,
                            ident_fn,
                            bias=c0f_sb[:, c, kp : kp + 1],
                            scale=c1f_sb[:, c, kp : kp + 1],
                        )
                stash[c] = (z, t1)

            def stage_b(c):
                z, t1 = stash.pop(c)
                t5 = tpool.tile([P, 2, B], F16, tag="t5")
                for h in range(2):
                    k0 = h * 16
                    # level 2: 16 -> 8 ; mult on DVE, add on PE into PSUM
                    p2 = tpool.tile([P, 8, B], F16, tag="p2")
                    nc.vector.tensor_mul(
                        p2,
                        z[:, 1, None, :].broadcast_to([P, 8, B]),
                        t1[:, k0 + 1 : k0 + 16 : 2, :],
                    )
                    t2 = psum.tile([P, 8, B], F32, tag="t2p")
                    t2f = t2[:].rearrange("p a b -> p (a b)")
                    t1ef = t1[:, k0 : k0 + 16 : 2, :]
                    p2f = p2[:].rearrange("p a b -> p (a b)")
                    for s in range(8):
                        sl = slice(s * B, (s + 1) * B)
                        nc.tensor.matmul(
                            t2f[:, sl], ident, t1ef[:, s, :], start=True, stop=False
                        )
                        nc.tensor.matmul(
                            t2f[:, sl], ident, p2f[:, sl], start=False, stop=True
                        )
                    # level 3: 8 -> 4 (reads fp32 PSUM at 1x)
                    p3 = tpool.tile([P, 4, B], F16, tag="p3")
                    nc.vector.tensor_mul(
                        p3, z[:, 2, None, :].broadcast_to([P, 4, B]), t2[:, 1::2, :]
                    )
                    t3 = tpool.tile([P, 4, B], F16, tag="t3")
                    nc.vector.tensor_add(t3, t2[:, 0::2, :], p3)
                    # level 4: 4 -> 2
                    p4 = tpool.tile([P, 2, B], F16, tag="p4")
                    nc.vector.tensor_mul(
                        p4, z[:, 3, None, :].broadcast_to([P, 2, B]), t3[:, 1::2, :]
                    )
                    t4 = tpool.tile([P, 2, B], F16, tag="t4")
                    nc.vector.tensor_add(t4, t3[:, 0::2, :], p4)
                    # level 5: 2 -> 1 on GpSimd
                    p5 = tpool.tile([P, B], F16, tag="p5")
                    nc.gpsimd.tensor_mul(p5, z[:, 4, :], t4[:, 1, :])
                    nc.gpsimd.tensor_add(t5[:, h, :], t4[:, 0, :], p5)
                # level 6: combine halves on GpSimd
                p6 = tpool.tile([P, B], F16, tag="p6")
                nc.gpsimd.tensor_mul(p6, z[:, 5, :], t5[:, 1, :])
                ot = tpool.tile([P, B], F16, tag="ot")
                nc.gpsimd.tensor_add(ot, t5[:, 0, :], p6)
                nc.sync.dma_start(outs[:, c, :], ot)

            for c in range(OHI + 1):
                if c < OHI:
                    stage_a(c)
                if c >= 1:
                    stage_b(c - 1)

    nc.compile()
    return nc


_CACHE: dict = {}


def _program():
    if "nc" not in _CACHE:
        _CACHE["nc"] = build_program()
    return _CACHE["nc"]


def _mobius(lut_table):
    """Per-bit (A,B) -> (A, B-A): c[o,m] = coefficient of
    prod_{j: bit_j(m)=1} x_{map(o,j)} in the multilinear expansion."""
    c = lut_table.astype(np.float64).reshape(OUT, *(2,) * NB)
    for ax in range(1, NB + 1):
        a = np.take(c, 0, axis=ax)
        b = np.take(c, 1, axis=ax)
        c = np.stack([a, b - a], axis=ax)
    return c.reshape(OUT, 1 << NB)


def make_inputs(x, lut_table, mapping):
    x = np.asarray(x, dtype=np.float32)
    lut_table = np.asarray(lut_table, dtype=np.float32)
    mapping = np.asarray(mapping)

    c = _mobius(lut_table)  # [OUT, 64], float64
    c0 = c[:, 0::2]  # even entries -> bias   [OUT, 32]
    c1 = c[:, 1::2]  # odd entries  -> scale  [OUT, 32]

    c0r = c0.reshape(NODE_SHARDS, OHI, P, 32)
    c1r = c1.reshape(NODE_SHARDS, OHI, P, 32)

    # gather indices: chunk-local position t = slot*128 + o_p, value =
    # mapping[o, slot]; wrapped into 16 partitions, tiled to 128
    m3 = mapping.reshape(NODE_SHARDS, OHI, P, NB)  # [ns, chunk, o_p, slot]
    tvals = np.transpose(m3, (0, 1, 3, 2)).reshape(NODE_SHARDS, -1)
    gidx_arrs = []
    for ns in range(NODE_SHARDS):
        g16 = tvals[ns].reshape(-1, 16).T.astype(np.int16)  # [16, OHI*IDXC]
        gidx_arrs.append(np.ascontiguousarray(np.tile(g16, (P // 16, 1))))

    gsrc_arrs = []
    for hb in range(BATCH_SHARDS):
        xh = x[hb * B : (hb + 1) * B]  # [B, IN]
        gsrc_arrs.append(np.ascontiguousarray(xh.T.astype(np.float16)))

    in_maps = []
    for core in range(N_CORES):
        ns, hb = core // BATCH_SHARDS, core % BATCH_SHARDS
        c0t = np.ascontiguousarray(np.transpose(c0r[ns], (1, 0, 2)))  # [P, OHI, 32]
        c1t = np.ascontiguousarray(np.transpose(c1r[ns], (1, 0, 2)))
        in_maps.append(
            {
                "gsrc": gsrc_arrs[hb],
                "gidx": gidx_arrs[ns],
                "c0f": c0t.astype(np.float32),
                "c1f": c1t.astype(np.float32),
                "c0d": np.ascontiguousarray(
                    np.repeat(c0t.astype(np.float16)[..., None], 2, axis=-1)
                ),
                "c1d": np.ascontiguousarray(
                    np.repeat(c1t.astype(np.float16)[..., None], 2, axis=-1)
                ),
            }
        )
    return in_maps


def assemble_output(results):
    """results: 8 dicts with 'outs' [P, OHI, B] fp16 -> full [B_FULL, OUT] f32."""
    full = np.empty((B_FULL, OUT), dtype=np.float32)
    for core in range(N_CORES):
        ns, hb = core // BATCH_SHARDS, core % BATCH_SHARDS
        arr = np.asarray(results[core]["outs"])  # [o_p, chunk, b]
        blk = arr.astype(np.float32).transpose(2, 1, 0).reshape(B, NODES)
        full[hb * B : (hb + 1) * B, ns * NODES : (ns + 1) * NODES] = blk
    return full


def kernel_with_results(x, lut_table, mapping, **kwargs):
    nc = _program()
    in_maps = make_inputs(x, lut_table, mapping)
    res = run_bass_kernel_spmd(nc, in_maps, core_ids=list(range(N_CORES)), **kwargs)
    return assemble_output(res.results), res


def kernel(x, lut_table, mapping):
    out, _ = kernel_with_results(x, lut_table, mapping)
    return out


if __name__ == "__main__":
    rng = np.random.default_rng(0)
    x = rng.random((B_FULL, IN), dtype=np.float32)
    lut = rng.standard_normal((OUT, 64), dtype=np.float32)
    mp = rng.integers(0, IN, (OUT, NB), dtype=np.int32)
    out = kernel(x, lut, mp)
    print(out.shape, out.dtype)


# revision 9
# speedup vs baseline: 1.2827x; 1.2827x over previous
"""Trainium2 Bass kernel for nn_BaseLUTLayer (soft-LUT layer).

Math: out[b,o] = sum_k lut[o,k] * prod_j (bit_j(k) ? x[b,m(o,j)] : 1-x[b,m(o,j)])

v3 strategy (Mobius / multilinear-polynomial basis):
  * Host re-parameterizes the LUT into multilinear coefficients c[o, :]
    (per-bit transform (A,B) -> (A, B-A)), so that
        out[b,o] = sum_m c[o,m] * prod_{j: bit_j(m)} x[b, map(o,j)]
    evaluated by a 6-level halving tree with ADJACENT pairing:
        t_l[i] = t_{l-1}[2i] + t_{l-1}[2i+1] * x_{map(o, l-1)}
    No 1-x / reciprocals / w-product; intermediates bounded by sum|c|
    (~500) so the whole pipeline runs in fp16.
  * Sharding: 4-way over nodes x 2-way over batch -> per core B=512
    batch rows, 512 nodes = 4 chunks of 128 nodes (nodes on partitions).
  * Gather: host passes x^T as [1024, 512] fp16 DRAM; dma_gather pulls
    6x128 rows (1KB) per chunk. No on-device transposes.
  * Engine split per chunk:
      - level 1 (32 kp-slices of scalar FMA): KACT slices on ScalarE
        (activation scale/bias), rest on DVE 2x via dup-pair trick
      - level 2 mult on DVE 2x; level-2 ADD on TensorE (identity matmul
        PSUM accumulation, fp16 moving)
      - level 3/4 on DVE (level 3 reads fp32 PSUM at 1x)
      - levels 5/6 on GpSimd (otherwise idle after gathers)
"""

import numpy as np

import concourse.bass as bass
import concourse.mybir as mybir
from concourse import bacc
from concourse import tile
from concourse.masks import make_identity
from concourse.bass_utils import run_bass_kernel_spmd

P = 128
IN = 1024
OUT = 2048
NB = 6
B_FULL = 1024
N_CORES = 8

NODE_SHARDS = 4
BATCH_SHARDS = 2
B = B_FULL // BATCH_SHARDS          # 512 batch rows per core
NODES = OUT // NODE_SHARDS          # 512 nodes per core
OHI = NODES // P                    # 4 chunks of 128 nodes

F16 = mybir.dt.float16
F32 = mybir.dt.float32
I16 = mybir.dt.int16

KACT = (13, 12)                     # level-1 kp slices per half on ScalarE
IDXC = NB * P // 16                 # gidx columns per chunk (48)


def build_program():
    nc = bacc.Bacc("TRN2", target_bir_lowering=False, debug=False)

    gsrc = nc.dram_tensor("gsrc", [IN, B], F16, kind="ExternalInput").ap()
    gidx = nc.dram_tensor("gidx", [P, OHI * IDXC], I16, kind="ExternalInput").ap()
    c0f = nc.dram_tensor("c0f", [P, OHI, 32], F32, kind="ExternalInput").ap()
    c1f = nc.dram_tensor("c1f", [P, OHI, 32], F32, kind="ExternalInput").ap()
    c0d = nc.dram_tensor("c0d", [P, OHI, 32, 2], F16, kind="ExternalInput").ap()
    c1d = nc.dram_tensor("c1d", [P, OHI, 32, 2], F16, kind="ExternalInput").ap()
    outs = nc.dram_tensor("outs", [P, OHI, B], F16, kind="ExternalOutput").ap()

    ident_fn = mybir.ActivationFunctionType.Identity

    with tile.TileContext(nc) as tc:
        with (
            tc.tile_pool(name="consts", bufs=1) as consts,
            tc.tile_pool(name="zpool", bufs=4) as zpool,
            tc.tile_pool(name="t1pool", bufs=2) as t1pool,
            tc.tile_pool(name="tpool", bufs=2) as tpool,
        ):
            gidx_sb = consts.tile([P, OHI * IDXC], I16)
            nc.sync.dma_start(gidx_sb, gidx)
            c0f_sb = consts.tile([P, OHI, 32], F32)
            nc.sync.dma_start(c0f_sb, c0f)
            c1f_sb = consts.tile([P, OHI, 32], F32)
            nc.sync.dma_start(c1f_sb, c1f)
            c0d_sb = consts.tile([P, OHI, 32, 2], F16)
            nc.sync.dma_start(c0d_sb, c0d)
            c1d_sb = consts.tile([P, OHI, 32, 2], F16)
            nc.sync.dma_start(c1d_sb, c1d)

            stash = {}

            def stage_a(c):
                z = zpool.tile([P, NB, B], F16, tag="z")
                if c == 0:
                    # slot-0 rows first so level 1 can start sooner
                    nc.gpsimd.dma_gather(
                        out_ap=z[:, 0:1, :],
                        in_ap=gsrc,
                        idxs_ap=gidx_sb[:, 0 : IDXC // NB],
                        num_idxs=P,
                        num_idxs_reg=P,
                        elem_size=B,
                    )
                    nc.gpsimd.dma_gather(
                        out_ap=z[:, 1:NB, :],
                        in_ap=gsrc,
                        idxs_ap=gidx_sb[:, IDXC // NB : IDXC],
                        num_idxs=(NB - 1) * P,
                        num_idxs_reg=(NB - 1) * P,
                        elem_size=B,
                    )
                else:
                    nc.gpsimd.dma_gather(
                        out_ap=z,
                        in_ap=gsrc,
                        idxs_ap=gidx_sb[:, c * IDXC : (c + 1) * IDXC],
                        num_idxs=NB * P,
                        num_idxs_reg=NB * P,
                        elem_size=B,
                    )
                z0 = z[:, 0, :]
                # level 1: t1[o, kp, b] = c0[o,kp] + c1[o,kp] * z0[o,b]
                t1 = t1pool.tile([P, 32, B], F16, tag="t1")
                z0d = z0.rearrange("p (b2 two) -> p b2 two", two=2)
                for h in range(2):
                    k0 = h * 16
                    kd0 = k0 + KACT[h]  # DVE takes the tail slices
                    kdve = 16 - KACT[h]
                    t1v = t1[:, kd0 : k0 + 16, :].rearrange(
                        "p k (b2 two) -> p k b2 two", two=2
                    )
                    in_z = z0d[:, None, :, :].broadcast_to([P, kdve, B // 2, 2])
                    nc.vector.tensor_mul(
                        t1v,
                        in_z,
                        c1d_sb[:, c, kd0 : k0 + 16, None, :].broadcast_to(
                            [P, kdve, B // 2, 2]
                        ),
                    )
                    nc.vector.tensor_add(
                        t1v,
                        t1v,
                        c0d_sb[:, c, kd0 : k0 + 16, None, :].broadcast_to(
                            [P, kdve, B // 2, 2]
                        ),
                    )
                    for kp in range(k0, kd0):
                        nc.scalar.activation(
                            t1[:, kp, :],
                            z0,
                            ident_fn,
                            bias=c0f_sb[:, c, kp : kp + 1],
                            scale=c1f_sb[:, c, kp : kp + 1],
                        )
                stash[c] = (z, t1)

            def stage_b(c):
                z, t1 = stash.pop(c)
                t5 = tpool.tile([P, 2, B], F16, tag="t5")
                for h in range(2):
                    k0 = h * 16
                    # level 2: 16 -> 8
                    p2 = tpool.tile([P, 8, B], F16, tag="p2")
                    nc.vector.tensor_mul(
                        p2,
                        z[:, 1, None, :].broadcast_to([P, 8, B]),
                        t1[:, k0 + 1 : k0 + 16 : 2, :],
                    )
                    t2 = tpool.tile([P, 8, B], F16, tag="t2")
                    nc.vector.tensor_add(t2, t1[:, k0 : k0 + 16 : 2, :], p2)
                    # level 3: 8 -> 4
                    p3 = tpool.tile([P, 4, B], F16, tag="p3")
                    nc.vector.tensor_mul(
                        p3, z[:, 2, None, :].broadcast_to([P, 4, B]), t2[:, 1::2, :]
                    )
                    t3 = tpool.tile([P, 4, B], F16, tag="t3")
                    nc.vector.tensor_add(t3, t2[:, 0::2, :], p3)
                    # level 4: 4 -> 2
                    p4 = tpool.tile([P, 2, B], F16, tag="p4")
                    nc.vector.tensor_mul(
                        p4, z[:, 3, None, :].broadcast_to([P, 2, B]), t3[:, 1::2, :]
                    )
                    t4 = tpool.tile([P, 2, B], F16, tag="t4")
                    nc.vector.tensor_add(t4, t3[:, 0::2, :], p4)
                    # level 5: 2 -> 1 on GpSimd
                    p5 = tpool.tile([P, B], F16, tag="p5")
                    nc.gpsimd.tensor_mul(p5, z[:, 4, :], t4[:, 1, :])
                    nc.gpsimd.tensor_add(t5[:, h, :], t4[:, 0, :], p5)
                # level 6: combine halves on GpSimd
                p6 = tpool.tile([P, B], F16, tag="p6")
                nc.gpsimd.tensor_mul(p6, z[:, 5, :], t5[:, 1, :])
                ot = tpool.tile([P, B], F16, tag="ot")
                nc.gpsimd.tensor_add(ot, t5[:, 0, :], p6)
                nc.sync.dma_start(outs[:, c, :], ot)

            for c in range(OHI + 1):
                if c < OHI:
                    stage_a(c)
                if c >= 1:
                    stage_b(c - 1)

    nc.compile()
    return nc


_CACHE: dict = {}


def _program():
    if "nc" not in _CACHE:
        _CACHE["nc"] = build_program()
    return _CACHE["nc"]


def _mobius(lut_table):
    """Per-bit (A,B) -> (A, B-A): c[o,m] = coefficient of
    prod_{j: bit_j(m)=1} x_{map(o,j)} in the multilinear expansion."""
    c = lut_table.astype(np.float64).reshape(OUT, *(2,) * NB)
    for ax in range(1, NB + 1):
        a = np.take(c, 0, axis=ax)
        b = np.take(c, 1, axis=ax)
        c = np.stack([a, b - a], axis=ax)
    return c.reshape(OUT, 1 << NB)


def make_inputs(x, lut_table, mapping):
    x = np.asarray(x, dtype=np.float32)
    lut_table = np.asarray(lut_table, dtype=np.float32)
    mapping = np.asarray(mapping)

    c = _mobius(lut_table)  # [OUT, 64], float64
    c0 = c[:, 0::2]  # even entries -> bias   [OUT, 32]
    c1 = c[:, 1::2]  # odd entries  -> scale  [OUT, 32]

    c0r = c0.reshape(NODE_SHARDS, OHI, P, 32)
    c1r = c1.reshape(NODE_SHARDS, OHI, P, 32)

    # gather indices: chunk-local position t = slot*128 + o_p, value =
    # mapping[o, slot]; wrapped into 16 partitions, tiled to 128
    m3 = mapping.reshape(NODE_SHARDS, OHI, P, NB)  # [ns, chunk, o_p, slot]
    tvals = np.transpose(m3, (0, 1, 3, 2)).reshape(NODE_SHARDS, -1)
    gidx_arrs = []
    for ns in range(NODE_SHARDS):
        g16 = tvals[ns].reshape(-1, 16).T.astype(np.int16)  # [16, OHI*IDXC]
        gidx_arrs.append(np.ascontiguousarray(np.tile(g16, (P // 16, 1))))

    gsrc_arrs = []
    for hb in range(BATCH_SHARDS):
        xh = x[hb * B : (hb + 1) * B]  # [B, IN]
        gsrc_arrs.append(np.ascontiguousarray(xh.T.astype(np.float16)))

    in_maps = []
    for core in range(N_CORES):
        ns, hb = core // BATCH_SHARDS, core % BATCH_SHARDS
        c0t = np.ascontiguousarray(np.transpose(c0r[ns], (1, 0, 2)))  # [P, OHI, 32]
        c1t = np.ascontiguousarray(np.transpose(c1r[ns], (1, 0, 2)))
        in_maps.append(
            {
                "gsrc": gsrc_arrs[hb],
                "gidx": gidx_arrs[ns],
                "c0f": c0t.astype(np.float32),
                "c1f": c1t.astype(np.float32),
                "c0d": np.ascontiguousarray(
                    np.repeat(c0t.astype(np.float16)[..., None], 2, axis=-1)
                ),
                "c1d": np.ascontiguousarray(
                    np.repeat(c1t.astype(np.float16)[..., None], 2, axis=-1)
                ),
            }
        )
    return in_maps


def assemble_output(results):
    """results: 8 dicts with 'outs' [P, OHI, B] fp16 -> full [B_FULL, OUT] f32."""
    full = np.empty((B_FULL, OUT), dtype=np.float32)
    for core in range(N_CORES):
        ns, hb = core // BATCH_SHARDS, core % BATCH_SHARDS
        arr = np.asarray(results[core]["outs"])  # [o_p, chunk, b]
        blk = arr.astype(np.float32).transpose(2, 1, 0).reshape(B, NODES)
        full[hb * B : (hb + 1) * B, ns * NODES : (ns + 1) * NODES] = blk
    return full


def kernel_with_results(x, lut_table, mapping, **kwargs):
    nc = _program()
    in_maps = make_inputs(x, lut_table, mapping)
    res = run_bass_kernel_spmd(nc, in_maps, core_ids=list(range(N_CORES)), **kwargs)
    return assemble_output(res.results), res


def kernel(x, lut_table, mapping):
    out, _ = kernel_with_results(x, lut_table, mapping)
    return out


if __name__ == "__main__":
    rng = np.random.default_rng(0)
    x = rng.random((B_FULL, IN), dtype=np.float32)
    lut = rng.standard_normal((OUT, 64), dtype=np.float32)
    mp = rng.integers(0, IN, (OUT, NB), dtype=np.int32)
    out = kernel(x, lut, mp)
    print(out.shape, out.dtype)


# revision 14
# speedup vs baseline: 1.3442x; 1.0479x over previous
"""Trainium2 Bass kernel for nn_BaseLUTLayer (soft-LUT layer).

Math: out[b,o] = sum_k lut[o,k] * prod_j (bit_j(k) ? x[b,m(o,j)] : 1-x[b,m(o,j)])

v3 strategy (Mobius / multilinear-polynomial basis):
  * Host re-parameterizes the LUT into multilinear coefficients c[o, :]
    (per-bit transform (A,B) -> (A, B-A)), so that
        out[b,o] = sum_m c[o,m] * prod_{j: bit_j(m)} x[b, map(o,j)]
    evaluated by a 6-level halving tree with ADJACENT pairing:
        t_l[i] = t_{l-1}[2i] + t_{l-1}[2i+1] * x_{map(o, l-1)}
    No 1-x / reciprocals / w-product; intermediates bounded by sum|c|
    (~500) so the whole pipeline runs in fp16.
  * Sharding: 4-way over nodes x 2-way over batch -> per core B=512
    batch rows, 512 nodes = 4 chunks of 128 nodes (nodes on partitions).
  * Gather: host passes x^T as [1024, 512] fp16 DRAM; dma_gather pulls
    6x128 rows (1KB) per chunk. No on-device transposes.
  * Engine split per chunk:
      - level 1 (32 kp-slices of scalar FMA): KACT slices on ScalarE
        (activation scale/bias), rest on DVE 2x via dup-pair trick
      - level 2 mult on DVE 2x; level-2 ADD on TensorE (identity matmul
        PSUM accumulation, fp16 moving)
      - level 3/4 on DVE (level 3 reads fp32 PSUM at 1x)
      - levels 5/6 on GpSimd (otherwise idle after gathers)
"""

import numpy as np

import concourse.bass as bass
import concourse.mybir as mybir
from concourse import bacc
from concourse import tile
from concourse.masks import make_identity
from concourse.bass_utils import run_bass_kernel_spmd

P = 128
IN = 1024
OUT = 2048
NB = 6
B_FULL = 1024
N_CORES = 8

NODE_SHARDS = 4
BATCH_SHARDS = 2
B = B_FULL // BATCH_SHARDS          # 512 batch rows per core
NODES = OUT // NODE_SHARDS          # 512 nodes per core
OHI = NODES // P                    # 4 chunks of 128 nodes

F16 = mybir.dt.float16
F32 = mybir.dt.float32
I16 = mybir.dt.int16

KACT = (14, 14)                     # level-1 kp slices per half on ScalarE
IDXC = NB * P // 16                 # gidx columns per chunk (48)


def build_program():
    nc = bacc.Bacc("TRN2", target_bir_lowering=False, debug=False)

    gsrc = nc.dram_tensor("gsrc", [IN, B], F16, kind="ExternalInput").ap()
    gidx = nc.dram_tensor("gidx", [P, OHI * IDXC], I16, kind="ExternalInput").ap()
    c0f = nc.dram_tensor("c0f", [P, OHI, 32], F32, kind="ExternalInput").ap()
    c1f = nc.dram_tensor("c1f", [P, OHI, 32], F32, kind="ExternalInput").ap()
    c0d = nc.dram_tensor("c0d", [P, OHI, 32, 2], F16, kind="ExternalInput").ap()
    c1d = nc.dram_tensor("c1d", [P, OHI, 32, 2], F16, kind="ExternalInput").ap()
    outs = nc.dram_tensor("outs", [P, OHI, B], F16, kind="ExternalOutput").ap()

    ident_fn = mybir.ActivationFunctionType.Identity

    with tile.TileContext(nc) as tc:
        with (
            tc.tile_pool(name="consts", bufs=1) as consts,
            tc.tile_pool(name="zpool", bufs=4) as zpool,
            tc.tile_pool(name="t1pool", bufs=3) as t1pool,
            tc.tile_pool(name="tpool", bufs=2) as tpool,
        ):
            # warm the dma_gather ucode (IRAM load ~10us) before gidx lands
            widx = consts.tile([P, 8], I16)
            nc.gpsimd.memset(widx, 0)
            warm = consts.tile([P, 1, B], F16)
            nc.gpsimd.dma_gather(
                out_ap=warm,
                in_ap=gsrc[0:1, :],
                idxs_ap=widx,
                num_idxs=P,
                num_idxs_reg=P,
                elem_size=B,
            )

            gidx_sb = consts.tile([P, OHI * IDXC], I16)
            nc.sync.dma_start(gidx_sb, gidx)
            c0f_sb = consts.tile([P, OHI, 32], F32)
            nc.sync.dma_start(c0f_sb, c0f)
            c1f_sb = consts.tile([P, OHI, 32], F32)
            nc.sync.dma_start(c1f_sb, c1f)
            c0d_sb = consts.tile([P, OHI, 32, 2], F16)
            nc.sync.dma_start(c0d_sb, c0d)
            c1d_sb = consts.tile([P, OHI, 32, 2], F16)
            nc.sync.dma_start(c1d_sb, c1d)

            stash = {}

            def stage_a(c):
                z = zpool.tile([P, NB, B], F16, tag="z")
                if c == 0:
                    # slot-0 rows first so level 1 can start sooner
                    nc.gpsimd.dma_gather(
                        out_ap=z[:, 0:1, :],
                        in_ap=gsrc,
                        idxs_ap=gidx_sb[:, 0 : IDXC // NB],
                        num_idxs=P,
                        num_idxs_reg=P,
                        elem_size=B,
                    )
                    nc.gpsimd.dma_gather(
                        out_ap=z[:, 1:NB, :],
                        in_ap=gsrc,
                        idxs_ap=gidx_sb[:, IDXC // NB : IDXC],
                        num_idxs=(NB - 1) * P,
                        num_idxs_reg=(NB - 1) * P,
                        elem_size=B,
                    )
                else:
                    nc.gpsimd.dma_gather(
                        out_ap=z,
                        in_ap=gsrc,
                        idxs_ap=gidx_sb[:, c * IDXC : (c + 1) * IDXC],
                        num_idxs=NB * P,
                        num_idxs_reg=NB * P,
                        elem_size=B,
                    )
                z0 = z[:, 0, :]
                # level 1: t1[o, kp, b] = c0[o,kp] + c1[o,kp] * z0[o,b]
                t1 = t1pool.tile([P, 32, B], F16, tag="t1")
                z0d = z0.rearrange("p (b2 two) -> p b2 two", two=2)
                for h in range(2):
                    k0 = h * 16
                    kd0 = k0 + KACT[h]  # DVE takes the tail slices
                    kdve = 16 - KACT[h]
                    t1v = t1[:, kd0 : k0 + 16, :].rearrange(
                        "p k (b2 two) -> p k b2 two", two=2
                    )
                    in_z = z0d[:, None, :, :].broadcast_to([P, kdve, B // 2, 2])
                    nc.vector.tensor_mul(
                        t1v,
                        in_z,
                        c1d_sb[:, c, kd0 : k0 + 16, None, :].broadcast_to(
                            [P, kdve, B // 2, 2]
                        ),
                    )
                    nc.vector.tensor_add(
                        t1v,
                        t1v,
                        c0d_sb[:, c, kd0 : k0 + 16, None, :].broadcast_to(
                            [P, kdve, B // 2, 2]
                        ),
                    )
                    for kp in range(k0, kd0):
                        nc.scalar.activation(
                            t1[:, kp, :],
                            z0,
                            ident_fn,
                            bias=c0f_sb[:, c, kp : kp + 1],
                            scale=c1f_sb[:, c, kp : kp + 1],
                        )
                stash[c] = (z, t1)

            def stage_b(c):
                z, t1 = stash.pop(c)
                # gpsimd level-5 ops cost ~2.7us each; keep the final chunk's
                # tail on DVE so the kernel end isn't gated on GpSimd
                eng5 = nc.gpsimd if c < OHI - 1 else nc.vector
                t5 = tpool.tile([P, 2, B], F16, tag="t5")
                for h in range(2):
                    k0 = h * 16
                    # level 2: 16 -> 8
                    p2 = tpool.tile([P, 8, B], F16, tag="p2")
                    nc.vector.tensor_mul(
                        p2,
                        z[:, 1, None, :].broadcast_to([P, 8, B]),
                        t1[:, k0 + 1 : k0 + 16 : 2, :],
                    )
                    t2 = tpool.tile([P, 8, B], F16, tag="t2")
                    nc.vector.tensor_add(t2, t1[:, k0 : k0 + 16 : 2, :], p2)
                    # level 3: 8 -> 4
                    p3 = tpool.tile([P, 4, B], F16, tag="p3")
                    nc.vector.tensor_mul(
                        p3, z[:, 2, None, :].broadcast_to([P, 4, B]), t2[:, 1::2, :]
                    )
                    t3 = tpool.tile([P, 4, B], F16, tag="t3")
                    nc.vector.tensor_add(t3, t2[:, 0::2, :], p3)
                    # level 4: 4 -> 2
                    p4 = tpool.tile([P, 2, B], F16, tag="p4")
                    nc.vector.tensor_mul(
                        p4, z[:, 3, None, :].broadcast_to([P, 2, B]), t3[:, 1::2, :]
                    )
                    t4 = tpool.tile([P, 2, B], F16, tag="t4")
                    nc.vector.tensor_add(t4, t3[:, 0::2, :], p4)
                    # level 5: 2 -> 1
                    p5 = tpool.tile([P, B], F16, tag="p5")
                    eng5.tensor_mul(p5, z[:, 4, :], t4[:, 1, :])
                    eng5.tensor_add(t5[:, h, :], t4[:, 0, :], p5)
                # level 6: combine halves on DVE
                p6 = tpool.tile([P, B], F16, tag="p6")
                nc.vector.tensor_mul(p6, z[:, 5, :], t5[:, 1, :])
                ot = tpool.tile([P, B], F16, tag="ot")
                nc.vector.tensor_add(ot, t5[:, 0, :], p6)
                nc.sync.dma_start(outs[:, c, :], ot)

            for c in range(OHI + 1):
                if c < OHI:
                    stage_a(c)
                if c >= 1:
                    stage_b(c - 1)

    nc.compile()
    return nc


_CACHE: dict = {}


def _program():
    if "nc" not in _CACHE:
        _CACHE["nc"] = build_program()
    return _CACHE["nc"]


def _mobius(lut_table):
    """Per-bit (A,B) -> (A, B-A): c[o,m] = coefficient of
    prod_{j: bit_j(m)=1} x_{map(o,j)} in the multilinear expansion."""
    c = lut_table.astype(np.float64).reshape(OUT, *(2,) * NB)
    for ax in range(1, NB + 1):
        a = np.take(c, 0, axis=ax)
        b = np.take(c, 1, axis=ax)
        c = np.stack([a, b - a], axis=ax)
    return c.reshape(OUT, 1 << NB)


def make_inputs(x, lut_table, mapping):
    x = np.asarray(x, dtype=np.float32)
    lut_table = np.asarray(lut_table, dtype=np.float32)
    mapping = np.asarray(mapping)

    c = _mobius(lut_table)  # [OUT, 64], float64
    c0 = c[:, 0::2]  # even entries -> bias   [OUT, 32]
    c1 = c[:, 1::2]  # odd entries  -> scale  [OUT, 32]

    c0r = c0.reshape(NODE_SHARDS, OHI, P, 32)
    c1r = c1.reshape(NODE_SHARDS, OHI, P, 32)

    # gather indices: chunk-local position t = slot*128 + o_p, value =
    # mapping[o, slot]; wrapped into 16 partitions, tiled to 128
    m3 = mapping.reshape(NODE_SHARDS, OHI, P, NB)  # [ns, chunk, o_p, slot]
    tvals = np.transpose(m3, (0, 1, 3, 2)).reshape(NODE_SHARDS, -1)
    gidx_arrs = []
    for ns in range(NODE_SHARDS):
        g16 = tvals[ns].reshape(-1, 16).T.astype(np.int16)  # [16, OHI*IDXC]
        gidx_arrs.append(np.ascontiguousarray(np.tile(g16, (P // 16, 1))))

    gsrc_arrs = []
    for hb in range(BATCH_SHARDS):
        xh = x[hb * B : (hb + 1) * B]  # [B, IN]
        gsrc_arrs.append(np.ascontiguousarray(xh.T.astype(np.float16)))

    in_maps = []
    for core in range(N_CORES):
        ns, hb = core // BATCH_SHARDS, core % BATCH_SHARDS
        c0t = np.ascontiguousarray(np.transpose(c0r[ns], (1, 0, 2)))  # [P, OHI, 32]
        c1t = np.ascontiguousarray(np.transpose(c1r[ns], (1, 0, 2)))
        in_maps.append(
            {
                "gsrc": gsrc_arrs[hb],
                "gidx": gidx_arrs[ns],
                "c0f": c0t.astype(np.float32),
                "c1f": c1t.astype(np.float32),
                "c0d": np.ascontiguousarray(
                    np.repeat(c0t.astype(np.float16)[..., None], 2, axis=-1)
                ),
                "c1d": np.ascontiguousarray(
                    np.repeat(c1t.astype(np.float16)[..., None], 2, axis=-1)
                ),
            }
        )
    return in_maps


def assemble_output(results):
    """results: 8 dicts with 'outs' [P, OHI, B] fp16 -> full [B_FULL, OUT] f32."""
    full = np.empty((B_FULL, OUT), dtype=np.float32)
    for core in range(N_CORES):
        ns, hb = core // BATCH_SHARDS, core % BATCH_SHARDS
        arr = np.asarray(results[core]["outs"])  # [o_p, chunk, b]
        blk = arr.astype(np.float32).transpose(2, 1, 0).reshape(B, NODES)
        full[hb * B : (hb + 1) * B, ns * NODES : (ns + 1) * NODES] = blk
    return full


def kernel_with_results(x, lut_table, mapping, **kwargs):
    nc = _program()
    in_maps = make_inputs(x, lut_table, mapping)
    res = run_bass_kernel_spmd(nc, in_maps, core_ids=list(range(N_CORES)), **kwargs)
    return assemble_output(res.results), res


def kernel(x, lut_table, mapping):
    out, _ = kernel_with_results(x, lut_table, mapping)
    return out


if __name__ == "__main__":
    rng = np.random.default_rng(0)
    x = rng.random((B_FULL, IN), dtype=np.float32)
    lut = rng.standard_normal((OUT, 64), dtype=np.float32)
    mp = rng.integers(0, IN, (OUT, NB), dtype=np.int32)
    out = kernel(x, lut, mp)
    print(out.shape, out.dtype)
